# revision 13
# baseline (speedup 1.0000x reference)
"""Trainium2 Bass kernel for nn_CustomLoss_87522843558003 (YOLO-style CIoU+BCE loss).

Strategy (data-parallel over batch, 8 cores):
 - Each core processes 8 consecutive batches. Its 8*8400 positions map onto
   128 SBUF partitions as [batch(8) x section(16)] rows of 525 positions,
   processed in 3 chunks of L=175 positions, then a batched 525-wide
   post-selection phase.
 - ACT does only Copy + one batched Ln per iteration (no table thrash):
     * arctan eliminated via atan(rt)-atan(rp) = atan(u),
       u = (wt*hp-wp*ht)/(hp*ht+wt*wp), and a rational fit
       F(z)=z(z+b)/(z^2+cz+d) ~ (4/pi^2) atan(sqrt(z))^2, z=u^2.
     * BCE via q = |p+t-1| (t in {0,1}): sum_c -ln q_c = -ln prod_c |d_c|,
       computed with one abs-multiply tensor_reduce + one Ln.
 - Anchor argmax via IoU cross-compare with fast reciprocal; selection with
   copy_predicated (first-max semantics).
 - All input DMA issued from the sync engine (SP) by default.
 - Per-partition masked sums via accum_out -> tiny [128,8] output per core;
   final normalization on host.
"""

import contextlib
import numpy as np

B, A, N, CH = 64, 3, 8400, 15
NCORES = 8
BPC = B // NCORES      # batches per core
SEC = 16               # partition sections per batch
PPART = BPC * SEC      # 128 partitions
POSROW = N // SEC      # 525 positions per partition row
NCHUNK = 3
L = POSROW // NCHUNK   # 175 positions per chunk per row
W5 = POSROW            # 525
NCLS = 10
EPS = 1e-7
# rational fit of (4/pi^2)*atan(sqrt(z))^2 (see session notes; end-loss err 6e-7)
FB = 18.5807497
FC = 29.74781457
FD = 47.19260109

_CACHE = {}


def _build_bass(loop_r=None, level=4, dma_engines=1, io_bufs=2, ss_bufs=2,
                per_bufs=1, w_bufs=1, b_bufs=1, sel6_act=True, d_stt=True):
    """loop_r: device-side For_i repeat count (None = single pass).
    level: staged build for perf attribution. -1=jumbo DMA probe,
    0=DMA+keepalive, 1=+argmax, 2=+selection+bce-prep, 3=+ciou, 4=full."""
    import concourse.tile as tile
    import concourse.mybir as mybir
    from concourse import bacc

    Alu = mybir.AluOpType
    Act = mybir.ActivationFunctionType
    f32 = mybir.dt.float32

    nc = bacc.Bacc("TRN2", target_bir_lowering=False, debug=False,
                   num_devices=NCORES)
    predL = nc.dram_tensor("predL", [BPC, A, N, CH], f32, kind="ExternalInput").ap()
    targL = nc.dram_tensor("targL", [BPC, N, CH], f32, kind="ExternalInput").ap()
    accO = nc.dram_tensor("acc_out", [PPART, 8], f32, kind="ExternalOutput").ap()

    pre = predL.rearrange("b a (s k j) c -> b a s k (j c)", s=SEC, k=NCHUNK, j=L)
    tre = targL.rearrange("b (s k j) c -> b s k (j c)", s=SEC, k=NCHUNK, j=L)
    FW = L * CH

    with tile.TileContext(nc) as tc:
        with (
            tc.tile_pool(name="pP", bufs=io_bufs) as pP,
            tc.tile_pool(name="pT", bufs=io_bufs) as pT,
            tc.tile_pool(name="pSS", bufs=ss_bufs) as pSS,
            tc.tile_pool(name="pPer", bufs=per_bufs) as pPer,
            tc.tile_pool(name="pW", bufs=w_bufs) as pW,
            tc.tile_pool(name="pB", bufs=b_bufs) as pB,
            tc.tile_pool(name="pAcc", bufs=1) as pAcc,
        ):
            ACC = pAcc.tile([PPART, 8], f32)
            deng = [nc.sync, nc.scalar, nc.gpsimd][:dma_engines]

            if level == -1:
                # jumbo DMA probe: whole-core loads, one DMA each, no compute
                PJ = pW.tile([PPART, BPC * A * N * CH // PPART], f32)
                TJ = pW.tile([PPART, BPC * N * CH // PPART], f32)
                CN = pW.tile([PPART, L], f32)
                loop_cmj = tc.For_i(0, loop_r, 1) if loop_r else contextlib.nullcontext()
                with loop_cmj:
                    nc.sync.dma_start(PJ[:], predL.rearrange("b a n c -> (b a n c)")
                                      .rearrange("(p f) -> p f", p=PPART))
                    deng[-1].dma_start(TJ[:], targL.rearrange("b n c -> (b n c)")
                                       .rearrange("(p f) -> p f", p=PPART))
                    nc.vector.tensor_scalar(CN[:], TJ[:, 0:L], 1.0, 0.0, Alu.mult,
                                            Alu.add, accum_out=ACC[:, 0:1])
                    nc.vector.tensor_scalar(CN[:], PJ[:, 0:L], 1.0, 0.0, Alu.mult,
                                            Alu.add, accum_out=ACC[:, 1:2])
                nc.sync.dma_start(accO, ACC[:])

            loop_cm = (tc.For_i(0, loop_r, 1)
                       if (loop_r and level != -1) else contextlib.nullcontext())
            with loop_cm:
              if level >= 0:
                # persistent per-iteration tiles (written per chunk, read batched)
                TLO5 = pPer.tile([PPART, W5 * 2], f32)
                THI5 = pPer.tile([PPART, W5 * 2], f32)
                TC2 = pPer.tile([PPART, W5 * 2], f32)
                TWH5 = pPer.tile([PPART, W5 * 2], f32)
                SEL6 = pPer.tile([PPART, W5 * 6], f32)
                LNIN = pPer.tile([PPART, W5 * 2], f32)   # [0:525]=obj, [525:]=clsprod
                MSK5 = pPer.tile([PPART, W5], f32)
                tlo5r = TLO5[:].rearrange("p (j c) -> p j c", c=2)
                thi5r = THI5[:].rearrange("p (j c) -> p j c", c=2)
                tc2r = TC2[:].rearrange("p (j c) -> p j c", c=2)
                twh5r = TWH5[:].rearrange("p (j c) -> p j c", c=2)
                sel6r = SEL6[:].rearrange("p (j c) -> p j c", c=6)

                for k in range(NCHUNK):
                    P = pP.tile([PPART, A * FW], f32)
                    T = pT.tile([PPART, FW], f32)
                    di = 0
                    for a in range(A):
                        deng[di % len(deng)].dma_start(
                            P[:, a * FW:(a + 1) * FW], pre[:, a, :, k, :])
                        di += 1
                    deng[di % len(deng)].dma_start(T[:], tre[:, :, k, :])
                    Pr = P[:].rearrange("p (a j c) -> p a j c", a=A, c=CH)
                    Tr = T[:].rearrange("p (j c) -> p j c", c=CH)
                    kL = slice(k * L, (k + 1) * L)
                    k2L = slice(k * 2 * L, (k + 1) * 2 * L)
                    k6L = slice(k * 6 * L, (k + 1) * 6 * L)

                    # ---- cnt accum + keepalive ----
                    CNT = pW.tile([PPART, L], f32)
                    nc.gpsimd.tensor_scalar(MSK5[:, kL], Tr[:, :, 4], 1.0, None,
                                            Alu.mult)
                    if level < 2:
                        nc.vector.tensor_scalar(CNT[:], MSK5[:, kL], 1.0, 0.0,
                                                Alu.mult, Alu.add,
                                                accum_out=ACC[:, 3:4])
                    if level == 0:
                        nc.vector.tensor_scalar(CNT[:], P[:, 0:L], 1.0, 0.0,
                                                Alu.mult, Alu.add,
                                                accum_out=ACC[:, 4:5])
                        continue

                    # ---- target prep ----
                    TWHH = pW.tile([PPART, L * 2], f32)
                    TA = pW.tile([PPART, L], f32)
                    twhhr = TWHH[:].rearrange("p (j c) -> p j c", c=2)
                    nc.gpsimd.tensor_scalar(twhhr, Tr[:, :, 2:4], 0.5, None, Alu.mult)
                    nc.gpsimd.tensor_tensor(tlo5r[:, kL], Tr[:, :, 0:2], twhhr,
                                            Alu.subtract)
                    nc.gpsimd.tensor_tensor(thi5r[:, kL], Tr[:, :, 0:2], twhhr,
                                            Alu.add)
                    nc.gpsimd.tensor_scalar(tc2r[:, kL], Tr[:, :, 0:2], 2.0, None,
                                            Alu.mult)
                    nc.gpsimd.tensor_scalar(twh5r[:, kL], Tr[:, :, 2:4], 1.0, None,
                                            Alu.mult)
                    nc.gpsimd.tensor_tensor(TA[:], Tr[:, :, 2], Tr[:, :, 3], Alu.mult)

                    # ---- per-anchor boxes + IoU pieces ----
                    # scratch reuse plan (write-after-last-read, dep-tracked):
                    #   PWHH: pwhh -> whr ; LT: lt -> whc ; RB: rb -> [s | q]
                    #   PA: pa -> ru
                    SS = pSS.tile([PPART, A * L * 6], f32)
                    SSr = SS[:].rearrange("p (a j c) -> p a j c", a=A, c=6)
                    PWHH = pW.tile([PPART, A * L * 2], f32)
                    LT = pW.tile([PPART, A * L * 2], f32)
                    RB = pW.tile([PPART, A * L * 2], f32)
                    PA = pW.tile([PPART, A * L], f32)
                    pwhhr = PWHH[:].rearrange("p (a j c) -> p a j c", a=A, c=2)
                    ltr = LT[:].rearrange("p (a j c) -> p a j c", a=A, c=2)
                    rbr = RB[:].rearrange("p (a j c) -> p a j c", a=A, c=2)
                    whrr = pwhhr
                    whcr = ltr
                    tlob = tlo5r[:, kL].unsqueeze(1).broadcast_to([PPART, A, L, 2])
                    thib = thi5r[:, kL].unsqueeze(1).broadcast_to([PPART, A, L, 2])

                    nc.gpsimd.tensor_scalar(pwhhr, Pr[:, :, :, 2:4], 0.5, None,
                                            Alu.mult)
                    nc.gpsimd.tensor_tensor(SSr[:, :, :, 0:2], Pr[:, :, :, 0:2],
                                            pwhhr, Alu.subtract)
                    nc.gpsimd.tensor_tensor(SSr[:, :, :, 2:4], Pr[:, :, :, 0:2],
                                            pwhhr, Alu.add)
                    nc.vector.tensor_tensor(ltr, SSr[:, :, :, 0:2], tlob, Alu.max)
                    nc.vector.tensor_tensor(rbr, SSr[:, :, :, 2:4], thib, Alu.min)
                    nc.gpsimd.tensor_tensor(whrr, rbr, ltr, Alu.subtract)
                    nc.vector.tensor_scalar(whcr, whrr, 0.0, None, Alu.max)

                    par = PA[:].rearrange("p (a j) -> p a j", a=A)
                    sr = RB[:, 0:A * L].rearrange("p (a j) -> p a j", a=A)
                    tab = TA[:].unsqueeze(1).broadcast_to([PPART, A, L])
                    nc.gpsimd.tensor_tensor(SSr[:, :, :, 4], whcr[:, :, :, 0],
                                            whcr[:, :, :, 1], Alu.mult)
                    nc.gpsimd.tensor_tensor(par, Pr[:, :, :, 2], Pr[:, :, :, 3],
                                            Alu.mult)
                    nc.gpsimd.tensor_tensor(sr, par, tab, Alu.add)
                    nc.gpsimd.tensor_tensor(SSr[:, :, :, 5], sr, SSr[:, :, :, 4],
                                            Alu.subtract)

                    # ---- argmax masks ----
                    rur = PA[:].rearrange("p (a j) -> p a j", a=A)
                    qr = RB[:, A * L:2 * A * L].rearrange("p (a j) -> p a j", a=A)
                    nc.vector.reciprocal_approx_fast(rur, SSr[:, :, :, 5])
                    nc.gpsimd.tensor_tensor(qr, SSr[:, :, :, 4], rur, Alu.mult)
                    G2 = pW.tile([PPART, 2 * L], f32)
                    G20 = pW.tile([PPART, L], f32)
                    W1M = pW.tile([PPART, L], f32)
                    W1 = pW.tile([PPART, L], f32)
                    W2 = pW.tile([PPART, L], f32)
                    g2r = G2[:].rearrange("p (g j) -> p g j", g=2)
                    nc.vector.tensor_tensor(g2r, qr[:, 1:3], qr[:, 0:2], Alu.is_gt)
                    nc.vector.tensor_tensor(G20[:], qr[:, 2], qr[:, 0], Alu.is_gt)
                    nc.gpsimd.tensor_tensor(W1M[:], g2r[:, 0], g2r[:, 1], Alu.mult)
                    nc.gpsimd.tensor_tensor(W1[:], g2r[:, 0], W1M[:], Alu.subtract)
                    nc.gpsimd.tensor_tensor(W2[:], G20[:], g2r[:, 1], Alu.mult)
                    if level < 2:
                        nc.vector.tensor_scalar(CNT[:], W1[:], 1.0, 0.0, Alu.mult,
                                                Alu.add, accum_out=ACC[:, 4:5])
                        nc.vector.tensor_scalar(CNT[:], W2[:], 1.0, 0.0, Alu.mult,
                                                Alu.add, accum_out=ACC[:, 5:6])
                        continue

                    # ---- selection (first-max semantics) ----
                    w1i = W1[:].bitcast(mybir.dt.int32)
                    w2i = W2[:].bitcast(mybir.dt.int32)
                    w1b6 = w1i.unsqueeze(2).broadcast_to([PPART, L, 6])
                    w2b6 = w2i.unsqueeze(2).broadcast_to([PPART, L, 6])
                    nc.vector.copy_predicated(SSr[:, 0], w1b6, SSr[:, 1])
                    nc.vector.copy_predicated(SSr[:, 0], w2b6, SSr[:, 2])
                    w1b11 = w1i.unsqueeze(2).broadcast_to([PPART, L, CH - 4])
                    w2b11 = w2i.unsqueeze(2).broadcast_to([PPART, L, CH - 4])
                    nc.vector.copy_predicated(Pr[:, 0, :, 4:CH], w1b11,
                                              Pr[:, 1, :, 4:CH])
                    nc.vector.copy_predicated(Pr[:, 0, :, 4:CH], w2b11,
                                              Pr[:, 2, :, 4:CH])

                    # stash selected box (+i,u) and obj; bce cls prep
                    if sel6_act:
                        nc.scalar.activation(SEL6[:, k6L], SS[:, 0:6 * L], Act.Copy)
                    else:
                        nc.gpsimd.tensor_scalar(SEL6[:, k6L], SS[:, 0:6 * L], 1.0,
                                                None, Alu.mult)
                    nc.gpsimd.tensor_scalar(LNIN[:, kL], Pr[:, 0, :, 4], 1.0, None,
                                            Alu.mult)
                    D = pW.tile([PPART, L * NCLS], f32)
                    Dr = D[:].rearrange("p (j c) -> p j c", c=NCLS)
                    if d_stt:
                        nc.vector.scalar_tensor_tensor(Dr, Pr[:, 0, :, 5:CH], -1.0,
                                                       Tr[:, :, 5:CH], Alu.add,
                                                       Alu.add)
                    else:
                        TM1 = pW.tile([PPART, L * NCLS], f32)
                        tm1r = TM1[:].rearrange("p (j c) -> p j c", c=NCLS)
                        nc.gpsimd.tensor_scalar(tm1r, Tr[:, :, 5:CH], 1.0, -1.0,
                                                Alu.mult, Alu.add)
                        nc.gpsimd.tensor_tensor(Dr, Pr[:, 0, :, 5:CH], tm1r, Alu.add)
                    nc.vector.tensor_reduce(
                        LNIN[:, W5 + k * L:W5 + (k + 1) * L], Dr,
                        mybir.AxisListType.X, Alu.mult, apply_absolute_value=True)

                if level == 0:
                    pass
                elif level >= 2:
                    # ---- batched post-selection phase (525-wide) ----
                    if level < 3:
                        DM = pB.tile([PPART, W5], f32)
                        nc.vector.tensor_scalar(DM[:], SEL6[:, 0:W5], 1.0, 0.0,
                                                Alu.mult, Alu.add,
                                                accum_out=ACC[:, 4:5])
                        nc.vector.tensor_scalar(DM[:], LNIN[:, W5:2 * W5], 1.0,
                                                0.0, Alu.mult, Alu.add,
                                                accum_out=ACC[:, 5:6])
                        nc.vector.tensor_scalar(DM[:], MSK5[:], 1.0, 0.0,
                                                Alu.mult, Alu.add,
                                                accum_out=ACC[:, 3:4])
                        nc.vector.tensor_scalar(DM[:], TC2[:, 0:W5], 1.0, 0.0,
                                                Alu.mult, Alu.add,
                                                accum_out=ACC[:, 6:7])
                        nc.vector.tensor_scalar(DM[:], TWH5[:, 0:W5], 1.0, 0.0,
                                                Alu.mult, Alu.add,
                                                accum_out=ACC[:, 7:8])
                    else:
                        # scratch tiles with manual lifetime-packed reuse:
                        # B1: clo -> spr -> sqcw      B2: chi -> dxy -> wh -> lnout
                        # B3: cw -> sqxy
                        # C1: riou -> diag -> n1 -> den2 -> vn -> aden -> dm
                        # C2: iou -> rdiag -> n2 -> rd2 -> vd0 -> v -> cioup
                        # C3: omie    C6: diou
                        # C4: cd -> num -> vd -> raden -> cnt5
                        # C5: qd -> d1 -> uu -> rvd -> v2 -> dm2
                        # C7: d2 -> z -> av -> dm3
                        B1 = pB.tile([PPART, W5 * 2], f32)
                        B2 = pB.tile([PPART, W5 * 2], f32)
                        B3 = pB.tile([PPART, W5 * 2], f32)
                        C1 = pB.tile([PPART, W5], f32)
                        C2 = pB.tile([PPART, W5], f32)
                        C3 = pB.tile([PPART, W5], f32)
                        C4 = pB.tile([PPART, W5], f32)
                        C5 = pB.tile([PPART, W5], f32)
                        C6 = pB.tile([PPART, W5], f32)
                        C7 = pB.tile([PPART, W5], f32)
                        b1r = B1[:].rearrange("p (j c) -> p j c", c=2)
                        b2r = B2[:].rearrange("p (j c) -> p j c", c=2)
                        b3r = B3[:].rearrange("p (j c) -> p j c", c=2)

                        nc.vector.reciprocal_approx_fast(C1[:], sel6r[:, :, 5])
                        nc.gpsimd.tensor_tensor(C2[:], sel6r[:, :, 4], C1[:],
                                                Alu.mult)                  # iou
                        nc.vector.tensor_scalar(C3[:], C2[:], -1.0, 1.0 + EPS,
                                                Alu.mult, Alu.add)         # omie
                        nc.vector.tensor_tensor(b1r, sel6r[:, :, 0:2], tlo5r,
                                                Alu.min)                   # clo
                        nc.vector.tensor_tensor(b2r, sel6r[:, :, 2:4], thi5r,
                                                Alu.max)                   # chi
                        nc.gpsimd.tensor_tensor(b3r, b2r, b1r, Alu.subtract)  # cw
                        nc.gpsimd.tensor_tensor(b1r, sel6r[:, :, 0:2],
                                                sel6r[:, :, 2:4], Alu.add)  # spr
                        nc.gpsimd.tensor_tensor(b2r, b1r, tc2r, Alu.subtract)  # dxy
                        nc.gpsimd.tensor_tensor(B1[:], B3[:], B3[:], Alu.mult)  # sqcw
                        nc.gpsimd.tensor_tensor(B3[:], B2[:], B2[:], Alu.mult)  # sqxy
                        nc.gpsimd.tensor_tensor(C1[:], b1r[:, :, 0], b1r[:, :, 1],
                                                Alu.add)                   # diag
                        nc.vector.reciprocal_approx_fast(C2[:], C1[:])     # rdiag
                        nc.gpsimd.tensor_tensor(C4[:], b3r[:, :, 0], b3r[:, :, 1],
                                                Alu.add)                   # cd
                        nc.gpsimd.tensor_tensor(C5[:], C4[:], C2[:], Alu.mult)  # qd
                        nc.vector.scalar_tensor_tensor(C6[:], C5[:], 0.25,
                                                       C3[:], Alu.mult, Alu.add)  # diou
                        # v-term via atan identity + rational fit
                        nc.gpsimd.tensor_tensor(b2r, sel6r[:, :, 2:4],
                                                sel6r[:, :, 0:2], Alu.subtract)  # wh
                        nc.gpsimd.tensor_tensor(C1[:], twh5r[:, :, 0],
                                                b2r[:, :, 1], Alu.mult)    # n1
                        nc.gpsimd.tensor_tensor(C2[:], b2r[:, :, 0],
                                                twh5r[:, :, 1], Alu.mult)  # n2
                        nc.gpsimd.tensor_tensor(C4[:], C1[:], C2[:],
                                                Alu.subtract)              # num
                        nc.gpsimd.tensor_tensor(C5[:], b2r[:, :, 1],
                                                twh5r[:, :, 1], Alu.mult)  # d1
                        nc.gpsimd.tensor_tensor(C7[:], b2r[:, :, 0],
                                                twh5r[:, :, 0], Alu.mult)  # d2
                        nc.gpsimd.tensor_tensor(C1[:], C5[:], C7[:], Alu.add)  # den2
                        nc.vector.reciprocal_approx_fast(C2[:], C1[:])     # rd2
                        nc.gpsimd.tensor_tensor(C5[:], C4[:], C2[:], Alu.mult)  # uu
                        nc.gpsimd.tensor_tensor(C7[:], C5[:], C5[:], Alu.mult)  # z
                        nc.vector.scalar_tensor_tensor(C1[:], C7[:], FB, C7[:],
                                                       Alu.add, Alu.mult)  # vn
                        nc.vector.scalar_tensor_tensor(C2[:], C7[:], FC, C7[:],
                                                       Alu.add, Alu.mult)  # vd0
                        nc.vector.tensor_scalar(C4[:], C2[:], 1.0, FD, Alu.mult,
                                                Alu.add)                   # vd
                        nc.vector.reciprocal_approx_fast(C5[:], C4[:])     # rvd
                        nc.gpsimd.tensor_tensor(C2[:], C1[:], C5[:], Alu.mult)  # v
                        nc.gpsimd.tensor_tensor(C1[:], C2[:], C3[:], Alu.add)  # aden
                        nc.vector.reciprocal_approx_fast(C4[:], C1[:])     # raden
                        nc.gpsimd.tensor_tensor(C5[:], C2[:], C2[:], Alu.mult)  # v2
                        nc.gpsimd.tensor_tensor(C7[:], C5[:], C4[:], Alu.mult)  # av
                        nc.gpsimd.tensor_tensor(C2[:], C6[:], C7[:], Alu.add)  # cioup

                        nc.vector.scalar_tensor_tensor(C1[:], C2[:], 1.0,
                                                       MSK5[:], Alu.mult, Alu.mult,
                                                       accum_out=ACC[:, 0:1])
                        nc.vector.tensor_scalar(C4[:], MSK5[:], 1.0, 0.0,
                                                Alu.mult, Alu.add,
                                                accum_out=ACC[:, 3:4])
                        if level == 3:
                            nc.vector.tensor_scalar(B2[:], LNIN[:], 1.0,
                                                    0.0, Alu.mult, Alu.add,
                                                    accum_out=ACC[:, 5:6])
                        if level >= 4:
                            nc.scalar.activation(B2[:], LNIN[:], Act.Ln)  # lnout
                            nc.vector.scalar_tensor_tensor(
                                C5[:], B2[:, 0:W5], 1.0, MSK5[:], Alu.mult,
                                Alu.mult, accum_out=ACC[:, 1:2])
                            nc.vector.scalar_tensor_tensor(
                                C7[:], B2[:, W5:2 * W5], 1.0, MSK5[:],
                                Alu.mult, Alu.mult, accum_out=ACC[:, 2:3])

            nc.sync.dma_start(accO, ACC[:])

    nc.compile()
    return nc


def kernel(pred, target):
    pred = np.ascontiguousarray(np.asarray(pred, dtype=np.float32))
    target = np.ascontiguousarray(np.asarray(target, dtype=np.float32))
    assert pred.shape == (B, A, N, CH) and target.shape == (B, N, CH)

    if "nc" not in _CACHE:
        _CACHE["nc"] = _build_bass()
    nc = _CACHE["nc"]

    from concourse import bass_utils

    in_maps = []
    for c in range(NCORES):
        lo, hi = c * BPC, (c + 1) * BPC
        in_maps.append({
            "predL": np.ascontiguousarray(pred[lo:hi]),
            "targL": np.ascontiguousarray(target[lo:hi]),
        })

    res = bass_utils.run_bass_kernel_spmd(nc, in_maps, core_ids=list(range(NCORES)))
    _CACHE["last_results"] = res

    per_batch = []
    for c in range(NCORES):
        acc = res.results[c]["acc_out"].astype(np.float32)   # [128, 8]
        num = acc[:, 0] - acc[:, 1] - 0.1 * acc[:, 2]        # ciou - obj_ln - 0.1*cls_ln
        cnt = acc[:, 3]
        nb = num.reshape(BPC, SEC).sum(axis=1, dtype=np.float32)
        cb = cnt.reshape(BPC, SEC).sum(axis=1, dtype=np.float32)
        per_batch.append(nb / cb)
    loss = np.mean(np.concatenate(per_batch), dtype=np.float32)
    return np.float32(loss)
